# revision 17
# baseline (speedup 1.0000x reference)
"""CenterPooling kernel for Trainium2 (8 NeuronCores, SPMD over batch).

Math note: for any tensor t, cummax(t, reverse=True) followed by cummax(t)
along the same axis equals broadcast(max(t, axis)) — the suffix-max is
non-increasing, so its prefix-max is the global max everywhere.  Hence:

    out[n,c,h,w] = A[n,c,h] + B[n,c,w]
    A = max_w relu(bn(conv3x3(x, w_up)))     (up branch)
    B = max_h relu(bn(conv3x3(x, w_down)))   (down branch)

BN folding: bn(y) = y*scale + shift with scale = g/sqrt(v+eps) per out
channel; scale folds into the conv weights on the host.  shift + relu are
monotone per channel, so they commute past the max and apply to the reduced
[C,H]/[C,W] tensors only.

Sharding: data-parallel over batch, 4 images per core, weights replicated.
"""

import sys

import numpy as np

for _p in ("/opt/trn_rl_repo", "/opt/pypackages"):
    if _p not in sys.path:
        sys.path.append(_p)

import concourse.bacc as bacc
import concourse.bass as bass
import concourse.mybir as mybir
import concourse.tile as tile
from concourse.bass_utils import run_bass_kernel_spmd

N_CORES = 8
B, C, H, W = 32, 256, 128, 128
BPC = B // N_CORES
EPS = 1e-5

F32 = mybir.dt.float32
BF16 = mybir.dt.bfloat16


def build_program(bpc: int = BPC, h: int = H, reps: int = 1) -> bass.Bass:
    """Build the per-core Bass program.

    Inputs (per core):
      x    [bpc, C, h, W] f32
      wq   [128, 2*2*9*C] bf16  packed conv weights (see pack_weights)
      bias [128, 4] f32         bn shifts per (branch, cout-tile)
    Output:
      out  [bpc, C, h, W] f32
    """
    assert h % 16 == 0
    WP = W + 2            # padded width  (zero cols at 0 and W+1)
    HP = h + 2            # padded height (zero rows at 0 and h+1)
    n_groups = h // 16    # 16 output rows per matmul group
    RELU = mybir.ActivationFunctionType.Relu
    AX = mybir.AxisListType.X

    nc = bacc.Bacc("TRN2", debug=False, enable_asserts=False)
    # x arrives pre-padded (1px zero border) and pre-cast to bf16 on the host
    x_d = nc.dram_tensor("x", [bpc, C, HP, WP], BF16, kind="ExternalInput")
    wq_d = nc.dram_tensor("wq", [128, 2 * 2 * 9 * C], BF16, kind="ExternalInput")
    bias_d = nc.dram_tensor("bias", [128, 4], F32, kind="ExternalInput")
    out_d = nc.dram_tensor("out", [bpc, C, h, W], F32, kind="ExternalOutput")
    xa, wa, ba, oa = x_d.ap(), wq_d.ap(), bias_d.ap(), out_d.ap()

    with tile.TileContext(nc) as tc:
        with (
            tc.tile_pool(name="wts", bufs=1) as wpool,
            tc.tile_pool(name="xpad", bufs=2) as xpool,
            tc.tile_pool(name="psum", bufs=8, space="PSUM") as ppool,
            tc.tile_pool(name="red", bufs=3) as rpool,
            tc.tile_pool(name="outp", bufs=3) as opool,
        ):
            wq_sb = wpool.tile([128, 2 * 2 * 9 * C], BF16, name="wq_sb")
            nc.sync.dma_start(wq_sb[:], wa[:, :])
            bias_sb = wpool.tile([128, 4], F32, name="bias_sb")
            nc.sync.dma_start(bias_sb[:], ba[:, :])

            for n_rep in range(bpc * reps):
                n = n_rep % bpc
                # ---- load image n: padded bf16 DRAM -> SBUF, one DMA per half ----
                xvs = []
                for ci in range(2):
                    xt = xpool.tile([128, HP * WP], BF16, tag=f"xp{ci}",
                                    name=f"xp{ci}_{n}")
                    xv = xt.rearrange("p (y x) -> p y x", x=WP)
                    nc.sync.dma_start(xv[:], xa[n, ci * 128:(ci + 1) * 128, :, :])
                    # PE touch: absorbs the DMA wait on the PE queue so the
                    # first real matmul stays within the 2-wait ISA limit
                    nc.tensor.ldweights(xv[:, 0, 0:128])
                    xvs.append(xv)

                # ---- conv branches; reduce to A[c,h] (up) / B[c,w] (down) ----
                # Each PSUM tile covers 3 whole padded rows: the matmul rhs is
                # ONE contiguous segment (measured ~270 ns/MM vs ~320 for a
                # 4-segment strided rhs).  The 2 pad columns per row yield
                # garbage outputs that the reduces never read.
                chunks = [(y0, min(3, h - y0)) for y0 in range(0, h, 3)]
                fins = {}
                for br in range(2):            # 0 = up, 1 = down
                    for co in range(2):        # cout tile
                        if br == 0:
                            acc = rpool.tile([128, h], F32, tag="Araw",
                                             name=f"Araw_{n}_{co}")
                        else:
                            acc = rpool.tile([128, W], F32, tag="Braw",
                                             name=f"Braw_{n}_{co}")
                            nc.vector.memset(acc[:], -3.0e38)
                        # groups of 8 chunks share one LDWEIGHTS per weight
                        # (the duplicate loads are deleted by _dedup_ldweights)
                        for g0 in range(0, len(chunks), 8):
                            grp = chunks[g0:g0 + 8]
                            pts = []
                            for y0, rows in grp:
                                pt = ppool.tile([128, 3, WP], F32, tag="ps",
                                                name=f"ps_{n}_{br}_{co}_{y0}")
                                pts.append(pt.rearrange("p a b -> p (a b)"))
                            for ci in range(2):
                                xf = xvs[ci].rearrange("p a b -> p (a b)")
                                for d in range(9):
                                    dy, dx = divmod(d, 3)
                                    woff = (br * 2 + ci) * (9 * C) + d * C + co * 128
                                    wap = wq_sb[:, woff:woff + 128]
                                    for k, (y0, rows) in enumerate(grp):
                                        nfree = (rows - 1) * WP + W
                                        off = (y0 + dy) * WP + dx
                                        nc.tensor.matmul(
                                            pts[k][:, 0:nfree], wap,
                                            xf[:, off:off + nfree],
                                            start=(ci == 0 and d == 0),
                                            stop=(ci == 1 and d == 8))
                            for k, (y0, rows) in enumerate(grp):
                                pv = pts[k].rearrange(
                                    "p (a b) -> p a b", b=WP)[:, 0:rows, 0:W]
                                if br == 0:
                                    # max over w within each row
                                    nc.vector.reduce_max(acc[:, y0:y0 + rows],
                                                         pv, axis=AX)
                                else:
                                    # max over rows per column, then running
                                    # max across row-chunks
                                    cm = rpool.tile([128, W], F32, tag="cm",
                                                    bufs=4,
                                                    name=f"cm_{n}_{co}_{y0}")
                                    nc.vector.reduce_max(
                                        cm[:], pv.transpose([0, 2, 1]), axis=AX)
                                    nc.vector.tensor_max(acc[:], acc[:], cm[:])
                        fin = rpool.tile([128, h if br == 0 else W], F32,
                                         tag="Af" if br == 0 else "Bf", bufs=4,
                                         name=f"fin_{n}_{br}_{co}")
                        bcol = br * 2 + co
                        nc.scalar.activation(fin[:], acc[:], RELU,
                                             bias=bias_sb[:, bcol:bcol + 1])
                        fins[(br, co)] = fin

                # ---- outer sum: out[c, y, x] = A[c, y] + B[c, x] ----
                HB = 8
                for co in range(2):
                    a_f = fins[(0, co)]
                    b_f = fins[(1, co)]
                    for hb in range(0, h, HB):
                        ot = opool.tile([128, HB, W], F32, tag="ot",
                                        name=f"ot_{n}_{co}_{hb}")
                        for j in range(HB):
                            nc.vector.tensor_scalar_add(
                                ot[:, j, :], b_f[:], a_f[:, hb + j:hb + j + 1])
                        nc.sync.dma_start(
                            oa[n, co * 128:(co + 1) * 128, hb:hb + HB, :], ot[:])
    _dedup_ldweights(nc)
    nc.compile()
    return nc


def _dedup_ldweights(nc) -> int:
    """Delete consecutive InstLdweights that reload identical weights.

    Tile lowering emits one LDWEIGHTS per matmul even when the stationary
    operand is unchanged; on HW the load serializes with streaming (~53 ns
    at FWL rate per MM).  The PE keeps the stationary operand between
    matmuls, and the non-self-loading InstMatmult still carries the weights
    AP in ins[1], so dropping an exact-duplicate reload is semantics
    preserving.  Only waits/updates-free duplicates are removed, and any
    other PE instruction resets the tracked state (conservative).
    """
    removed = 0
    for bb in nc.m.functions[0].blocks:
        last_key = None
        to_remove = []
        for inst in bb.instructions:
            tn = type(inst).__name__
            if getattr(inst, "engine", None) != mybir.EngineType.PE:
                continue
            if tn == "InstLdweights":
                si = inst.sync_info
                clean = si is None or (not si.on_wait and not si.on_update)
                key = (inst.pretty_str() if hasattr(inst, "pretty_str")
                       else repr(inst.ins[0]))
                key = repr(inst.ins[0])
                if clean and last_key == key:
                    to_remove.append(inst)
                    removed += 1
                else:
                    last_key = key
            elif tn == "InstMatmult":
                continue  # non-self-loading: weights state unchanged
            else:
                last_key = None
        for inst in to_remove:
            bb.instructions.remove(inst)
    return removed


def pack_weights(w: np.ndarray, gamma: np.ndarray, var: np.ndarray) -> np.ndarray:
    """Fold BN scale into OIHW conv weights, emit bf16 lhsT layout.

    Output [128, 2*9*256]: free index = ci_t*(9*256) + (ky*3+kx)*256 + co,
    partition = ci % 128.  lhsT slice [:, off:off+128] is then [K=ci, M=co]
    for one (ci_t, tap, co_t).
    """
    import ml_dtypes
    scale = gamma / np.sqrt(var + EPS)
    wf = (np.asarray(w, np.float32) * scale[:, None, None, None]).astype(np.float32)
    wt = np.transpose(wf, (1, 2, 3, 0))          # [I, ky, kx, O]
    wt = wt.reshape(2, 128, 9, C)                # [ci_t, ci_p, tap, O]
    wt = np.transpose(wt, (1, 0, 2, 3))          # [ci_p, ci_t, tap, O]
    return np.ascontiguousarray(wt.reshape(128, 2 * 9 * C)).astype(ml_dtypes.bfloat16)


def pack_x(x: np.ndarray) -> np.ndarray:
    """Zero-pad spatial dims by 1px and cast to bf16 (RNE, same as on-chip)."""
    import ml_dtypes
    n, c, h, w = x.shape
    xp = np.zeros((n, c, h + 2, w + 2), dtype=ml_dtypes.bfloat16)
    xp[:, :, 1:h + 1, 1:w + 1] = x.astype(ml_dtypes.bfloat16)
    return xp


def pack_bias(b_up, m_up, g_up, v_up, b_down, m_down, g_down, v_down) -> np.ndarray:
    def shift(b, m, g, v):
        return b - m * (g / np.sqrt(v + EPS))
    s_up = shift(b_up, m_up, g_up, v_up).astype(np.float32)
    s_dn = shift(b_down, m_down, g_down, v_down).astype(np.float32)
    return np.ascontiguousarray(
        np.stack([s_up[:128], s_up[128:], s_dn[:128], s_dn[128:]], axis=1))


_CACHE: dict = {}


def _get_program() -> bass.Bass:
    if "nc" not in _CACHE:
        _CACHE["nc"] = build_program()
    return _CACHE["nc"]


def make_in_maps(x, w_up, g_up, b_up, m_up, v_up,
                 w_down, g_down, b_down, m_down, v_down):
    x = pack_x(np.asarray(x, dtype=np.float32))
    wq = np.concatenate(
        [pack_weights(np.asarray(w_up, np.float32), np.asarray(g_up, np.float32),
                      np.asarray(v_up, np.float32)),
         pack_weights(np.asarray(w_down, np.float32), np.asarray(g_down, np.float32),
                      np.asarray(v_down, np.float32))], axis=1)
    bias = pack_bias(np.asarray(b_up, np.float32), np.asarray(m_up, np.float32),
                     np.asarray(g_up, np.float32), np.asarray(v_up, np.float32),
                     np.asarray(b_down, np.float32), np.asarray(m_down, np.float32),
                     np.asarray(g_down, np.float32), np.asarray(v_down, np.float32))
    return [{"x": x[i * BPC:(i + 1) * BPC], "wq": wq, "bias": bias}
            for i in range(N_CORES)]


def execute(in_maps):
    nc = _get_program()
    return run_bass_kernel_spmd(nc, in_maps, list(range(N_CORES)))


def kernel(x, w_up, g_up, b_up, m_up, v_up,
           w_down, g_down, b_down, m_down, v_down) -> np.ndarray:
    in_maps = make_in_maps(x, w_up, g_up, b_up, m_up, v_up,
                           w_down, g_down, b_down, m_down, v_down)
    res = execute(in_maps)
    return np.concatenate([res.results[i]["out"] for i in range(N_CORES)], axis=0)


# revision 22
# speedup vs baseline: 1.0032x; 1.0032x over previous
"""CenterPooling kernel for Trainium2 (8 NeuronCores, SPMD over batch).

Math note: for any tensor t, cummax(t, reverse=True) followed by cummax(t)
along the same axis equals broadcast(max(t, axis)) — the suffix-max is
non-increasing, so its prefix-max is the global max everywhere.  Hence:

    out[n,c,h,w] = A[n,c,h] + B[n,c,w]
    A = max_w relu(bn(conv3x3(x, w_up)))     (up branch)
    B = max_h relu(bn(conv3x3(x, w_down)))   (down branch)

BN folding: bn(y) = y*scale + shift with scale = g/sqrt(v+eps) per out
channel; scale folds into the conv weights on the host.  shift + relu are
monotone per channel, so they commute past the max and apply to the reduced
[C,H]/[C,W] tensors only.

Sharding: data-parallel over batch, 4 images per core, weights replicated.
"""

import sys

import numpy as np

for _p in ("/opt/trn_rl_repo", "/opt/pypackages"):
    if _p not in sys.path:
        sys.path.append(_p)

import concourse.bacc as bacc
import concourse.bass as bass
import concourse.mybir as mybir
import concourse.tile as tile
from concourse.bass_utils import run_bass_kernel_spmd

N_CORES = 8
B, C, H, W = 32, 256, 128, 128
BPC = B // N_CORES
EPS = 1e-5

F32 = mybir.dt.float32
BF16 = mybir.dt.bfloat16


def build_program(bpc: int = BPC, h: int = H, reps: int = 1, grp: int = 4,
                  tmax_gpsimd: bool = False) -> bass.Bass:
    # tmax_gpsimd stays False: walrus codegen rejects TensorTensor on Pool
    """Build the per-core Bass program.

    Inputs (per core):
      x    [bpc, C, h, W] f32
      wq   [128, 2*2*9*C] bf16  packed conv weights (see pack_weights)
      bias [128, 4] f32         bn shifts per (branch, cout-tile)
    Output:
      out  [bpc, C, h, W] f32
    """
    assert h % 16 == 0
    WP = W + 2            # padded width  (zero cols at 0 and W+1)
    HP = h + 2            # padded height (zero rows at 0 and h+1)
    n_groups = h // 16    # 16 output rows per matmul group
    RELU = mybir.ActivationFunctionType.Relu
    AX = mybir.AxisListType.X

    nc = bacc.Bacc("TRN2", debug=False, enable_asserts=False)
    # x arrives pre-padded (1px zero border) and pre-cast to bf16 on the host
    x_d = nc.dram_tensor("x", [bpc, C, HP, WP], BF16, kind="ExternalInput")
    wq_d = nc.dram_tensor("wq", [128, 2 * 2 * 9 * C], BF16, kind="ExternalInput")
    bias_d = nc.dram_tensor("bias", [128, 4], F32, kind="ExternalInput")
    out_d = nc.dram_tensor("out", [bpc, C, h, W], F32, kind="ExternalOutput")
    xa, wa, ba, oa = x_d.ap(), wq_d.ap(), bias_d.ap(), out_d.ap()

    with tile.TileContext(nc) as tc:
        with (
            tc.tile_pool(name="wts", bufs=1) as wpool,
            tc.tile_pool(name="xpad", bufs=2) as xpool,
            tc.tile_pool(name="psum", bufs=8, space="PSUM") as ppool,
            tc.tile_pool(name="red", bufs=3) as rpool,
            tc.tile_pool(name="outp", bufs=3) as opool,
        ):
            wq_sb = wpool.tile([128, 2 * 2 * 9 * C], BF16, name="wq_sb")
            nc.sync.dma_start(wq_sb[:], wa[:, :])
            bias_sb = wpool.tile([128, 4], F32, name="bias_sb")
            nc.sync.dma_start(bias_sb[:], ba[:, :])

            for n_rep in range(bpc * reps):
                n = n_rep % bpc
                # ---- load image n: padded bf16 DRAM -> SBUF, one DMA per half ----
                xvs = []
                for ci in range(2):
                    xt = xpool.tile([128, HP * WP], BF16, tag=f"xp{ci}",
                                    name=f"xp{ci}_{n}")
                    xv = xt.rearrange("p (y x) -> p y x", x=WP)
                    nc.sync.dma_start(xv[:], xa[n, ci * 128:(ci + 1) * 128, :, :])
                    # PE touch: absorbs the DMA wait on the PE queue so the
                    # first real matmul stays within the 2-wait ISA limit
                    nc.tensor.ldweights(xv[:, 0, 0:128])
                    xvs.append(xv)

                # ---- conv branches; reduce to A[c,h] (up) / B[c,w] (down) ----
                # Each PSUM tile covers 3 whole padded rows: the matmul rhs is
                # ONE contiguous segment (measured ~270 ns/MM vs ~320 for a
                # 4-segment strided rhs).  The 2 pad columns per row yield
                # garbage outputs that the reduces never read.
                chunks = [(y0, min(3, h - y0)) for y0 in range(0, h, 3)]
                fins = {}
                for br in range(2):            # 0 = up, 1 = down
                    for co in range(2):        # cout tile
                        if br == 0:
                            acc = rpool.tile([128, h], F32, tag="Araw",
                                             name=f"Araw_{n}_{co}")
                        else:
                            acc = rpool.tile([128, W], F32, tag="Braw",
                                             name=f"Braw_{n}_{co}")
                            nc.vector.memset(acc[:], -3.0e38)
                        # chunk groups share one LDWEIGHTS per weight (the
                        # duplicate loads are deleted by _dedup_ldweights);
                        # grp=4 of the 8 PSUM banks keeps two groups in
                        # flight so the end-of-group DVE reduce burst hides
                        # under the next group's matmuls
                        for g0 in range(0, len(chunks), grp):
                            cgrp = chunks[g0:g0 + grp]
                            pts = []
                            for y0, rows in cgrp:
                                pt = ppool.tile([128, 3, WP], F32, tag="ps",
                                                name=f"ps_{n}_{br}_{co}_{y0}")
                                pts.append(pt.rearrange("p a b -> p (a b)"))
                            for ci in range(2):
                                xf = xvs[ci].rearrange("p a b -> p (a b)")
                                for d in range(9):
                                    dy, dx = divmod(d, 3)
                                    woff = (br * 2 + ci) * (9 * C) + d * C + co * 128
                                    wap = wq_sb[:, woff:woff + 128]
                                    for k, (y0, rows) in enumerate(cgrp):
                                        nfree = (rows - 1) * WP + W
                                        off = (y0 + dy) * WP + dx
                                        nc.tensor.matmul(
                                            pts[k][:, 0:nfree], wap,
                                            xf[:, off:off + nfree],
                                            start=(ci == 0 and d == 0),
                                            stop=(ci == 1 and d == 8))
                            for k, (y0, rows) in enumerate(cgrp):
                                pv = pts[k].rearrange(
                                    "p (a b) -> p a b", b=WP)[:, 0:rows, 0:W]
                                if br == 0:
                                    # max over w within each row
                                    nc.vector.reduce_max(acc[:, y0:y0 + rows],
                                                         pv, axis=AX)
                                else:
                                    # max over rows per column, then running
                                    # max across row-chunks
                                    cm = rpool.tile([128, W], F32, tag="cm",
                                                    bufs=4,
                                                    name=f"cm_{n}_{co}_{y0}")
                                    nc.vector.reduce_max(
                                        cm[:], pv.transpose([0, 2, 1]), axis=AX)
                                    eng = nc.gpsimd if tmax_gpsimd else nc.vector
                                    eng.tensor_max(acc[:], acc[:], cm[:])
                        fin = rpool.tile([128, h if br == 0 else W], F32,
                                         tag="Af" if br == 0 else "Bf", bufs=4,
                                         name=f"fin_{n}_{br}_{co}")
                        bcol = br * 2 + co
                        nc.scalar.activation(fin[:], acc[:], RELU,
                                             bias=bias_sb[:, bcol:bcol + 1])
                        fins[(br, co)] = fin

                # ---- outer sum: out[c, y, x] = A[c, y] + B[c, x] ----
                HB = 8
                for co in range(2):
                    a_f = fins[(0, co)]
                    b_f = fins[(1, co)]
                    for hb in range(0, h, HB):
                        ot = opool.tile([128, HB, W], F32, tag="ot",
                                        name=f"ot_{n}_{co}_{hb}")
                        for j in range(HB):
                            nc.vector.tensor_scalar_add(
                                ot[:, j, :], b_f[:], a_f[:, hb + j:hb + j + 1])
                        nc.sync.dma_start(
                            oa[n, co * 128:(co + 1) * 128, hb:hb + HB, :], ot[:])
    _dedup_ldweights(nc)
    nc.compile()
    return nc


def _dedup_ldweights(nc) -> int:
    """Delete consecutive InstLdweights that reload identical weights.

    Tile lowering emits one LDWEIGHTS per matmul even when the stationary
    operand is unchanged; on HW the load serializes with streaming (~53 ns
    at FWL rate per MM).  The PE keeps the stationary operand between
    matmuls, and the non-self-loading InstMatmult still carries the weights
    AP in ins[1], so dropping an exact-duplicate reload is semantics
    preserving.  Only waits/updates-free duplicates are removed, and any
    other PE instruction resets the tracked state (conservative).
    """
    removed = 0
    for bb in nc.m.functions[0].blocks:
        last_key = None
        keep = []
        for inst in bb.instructions:
            tn = type(inst).__name__
            if getattr(inst, "engine", None) == mybir.EngineType.PE:
                if tn == "InstLdweights":
                    si = inst.sync_info
                    clean = si is None or (not si.on_wait and not si.on_update)
                    key = repr(inst.ins[0])
                    if clean and last_key == key:
                        removed += 1
                        continue  # drop exact-duplicate reload
                    last_key = key
                elif tn != "InstMatmult":
                    # unknown PE instruction: assume weights state clobbered
                    last_key = None
            keep.append(inst)
        bb.instructions[:] = keep
    return removed


def pack_weights(w: np.ndarray, gamma: np.ndarray, var: np.ndarray) -> np.ndarray:
    """Fold BN scale into OIHW conv weights, emit bf16 lhsT layout.

    Output [128, 2*9*256]: free index = ci_t*(9*256) + (ky*3+kx)*256 + co,
    partition = ci % 128.  lhsT slice [:, off:off+128] is then [K=ci, M=co]
    for one (ci_t, tap, co_t).
    """
    import ml_dtypes
    scale = gamma / np.sqrt(var + EPS)
    wf = (np.asarray(w, np.float32) * scale[:, None, None, None]).astype(np.float32)
    wt = np.transpose(wf, (1, 2, 3, 0))          # [I, ky, kx, O]
    wt = wt.reshape(2, 128, 9, C)                # [ci_t, ci_p, tap, O]
    wt = np.transpose(wt, (1, 0, 2, 3))          # [ci_p, ci_t, tap, O]
    return np.ascontiguousarray(wt.reshape(128, 2 * 9 * C)).astype(ml_dtypes.bfloat16)


def pack_x(x: np.ndarray) -> np.ndarray:
    """Zero-pad spatial dims by 1px and cast to bf16 (RNE, same as on-chip)."""
    import ml_dtypes
    n, c, h, w = x.shape
    xp = np.zeros((n, c, h + 2, w + 2), dtype=ml_dtypes.bfloat16)
    xp[:, :, 1:h + 1, 1:w + 1] = x.astype(ml_dtypes.bfloat16)
    return xp


def pack_bias(b_up, m_up, g_up, v_up, b_down, m_down, g_down, v_down) -> np.ndarray:
    def shift(b, m, g, v):
        return b - m * (g / np.sqrt(v + EPS))
    s_up = shift(b_up, m_up, g_up, v_up).astype(np.float32)
    s_dn = shift(b_down, m_down, g_down, v_down).astype(np.float32)
    return np.ascontiguousarray(
        np.stack([s_up[:128], s_up[128:], s_dn[:128], s_dn[128:]], axis=1))


_CACHE: dict = {}


def _get_program() -> bass.Bass:
    if "nc" not in _CACHE:
        _CACHE["nc"] = build_program()
    return _CACHE["nc"]


def make_in_maps(x, w_up, g_up, b_up, m_up, v_up,
                 w_down, g_down, b_down, m_down, v_down):
    x = pack_x(np.asarray(x, dtype=np.float32))
    wq = np.concatenate(
        [pack_weights(np.asarray(w_up, np.float32), np.asarray(g_up, np.float32),
                      np.asarray(v_up, np.float32)),
         pack_weights(np.asarray(w_down, np.float32), np.asarray(g_down, np.float32),
                      np.asarray(v_down, np.float32))], axis=1)
    bias = pack_bias(np.asarray(b_up, np.float32), np.asarray(m_up, np.float32),
                     np.asarray(g_up, np.float32), np.asarray(v_up, np.float32),
                     np.asarray(b_down, np.float32), np.asarray(m_down, np.float32),
                     np.asarray(g_down, np.float32), np.asarray(v_down, np.float32))
    return [{"x": x[i * BPC:(i + 1) * BPC], "wq": wq, "bias": bias}
            for i in range(N_CORES)]


def execute(in_maps):
    nc = _get_program()
    return run_bass_kernel_spmd(nc, in_maps, list(range(N_CORES)))


def kernel(x, w_up, g_up, b_up, m_up, v_up,
           w_down, g_down, b_down, m_down, v_down) -> np.ndarray:
    in_maps = make_in_maps(x, w_up, g_up, b_up, m_up, v_up,
                           w_down, g_down, b_down, m_down, v_down)
    res = execute(in_maps)
    return np.concatenate([res.results[i]["out"] for i in range(N_CORES)], axis=0)
